# revision 1
# baseline (speedup 1.0000x reference)
"""Trainium2 Bass kernel for nn_DynamLinear: per-codebook linear -> chunked
outer product -> mean over codebooks -> RMS norm.

Math notes:
  ref: y = einsum('td,hdo->tho', x, W); split o=64 into a=y[..., :32], b=y[..., 32:]
       op[t,h,i,j] = a[t,h,i]*b[t,h,j];  out = mean_h(op)*sqrt(16); rms_norm(out)
  Since rms_norm is scale invariant, out = S / sqrt(mean(S^2) + 16e-12) where
       S[t,i,j] = sum_h a[t,h,i]*b[t,h,j]  (the per-token 16x32^T @ 16x32 matmul)

Per-core plan (tokens sharded 1024/core):
  stage1: y^T = Wp^T @ x^T on TensorE (bf16), columns ordered so that a
          SBUF->SBUF DMA "shuffle" lands y into z[32r+h, sel, i, t256]
          (r = token/256, sel = a/b, t256 = token%256).
  stage2: per token one self-loading matmul lhsT=A_t[16h x 32i],
          rhs=B_t[16h x 32j] on a 32x32 PE tile (row group r, col group
          c = token%4) -> PSUM S_t[32i x 32j].
  rms:    ACT square, DVE reduce over j, indicator-matmul reduces over i
          (and broadcasts the per-token sums to all 128 partitions),
          sqrt+reciprocal, DVE multiply, strided DMA store.
"""

import os
import sys
import functools
from contextlib import ExitStack

import numpy as np
import ml_dtypes

if "/opt/trn_rl_repo" not in sys.path:
    sys.path.insert(0, "/opt/trn_rl_repo")

import concourse.bass as bass
import concourse.bacc as bacc
import concourse.tile as tile
from concourse import mybir
from concourse.bass_utils import run_bass_kernel_spmd

N_CORES = 8
T_CORE = 1024          # tokens per core
D = 1024               # feat dim
H = 16                 # codebooks
EPS = 16e-12           # 16 * 1e-12 (scale-folded reference eps)

F32 = mybir.dt.float32
F32R = mybir.dt.float32r
BF16 = mybir.dt.bfloat16


def _kernel_body(tc, ctx, xt, wp, ind, out):
    nc = tc.nc

    singles = ctx.enter_context(tc.tile_pool(name="singles", bufs=1))
    psum1 = ctx.enter_context(tc.tile_pool(name="psum1", bufs=3, space="PSUM"))
    psum2 = ctx.enter_context(tc.tile_pool(name="psum2", bufs=4, space="PSUM"))
    psum3 = ctx.enter_context(tc.tile_pool(name="psum3", bufs=1, space="PSUM"))
    scratch = ctx.enter_context(tc.tile_pool(name="scratch", bufs=2))
    smalls = ctx.enter_context(tc.tile_pool(name="smalls", bufs=4))

    # ---- resident inputs -------------------------------------------------
    wp_sb = singles.tile([128, 8, 8, 128], BF16)  # [dp, CT, dt, c7]
    xt_sb = singles.tile([128, 8, 1024], BF16)    # [dp, dt, t]: x^T
    nc.sync.dma_start(out=wp_sb[:, 0], in_=wp[:, 0])
    nc.sync.dma_start(out=xt_sb[:, :, 0:256], in_=xt[:, :, 0:256])
    for CT in range(1, 8):
        nc.sync.dma_start(out=wp_sb[:, CT], in_=wp[:, CT])
    for q in range(1, 4):
        nc.sync.dma_start(out=xt_sb[:, :, q * 256:(q + 1) * 256],
                          in_=xt[:, :, q * 256:(q + 1) * 256])
    ind_sb = singles.tile([128, 128], F32R)       # block-diag ones (4x 32x32)
    nc.sync.dma_start(out=ind_sb[:], in_=ind[:])
    eps_sb = singles.tile([128, 1], F32)
    nc.vector.memset(eps_sb[:], EPS)

    # ---- pipelined over token quarters tq (= row group r) ----------------
    # stage 1: y^T[col, t] for quarter tq -> shuffle(tq) -> stage2 chunks
    y_sb = singles.tile([128, 8, 1024], BF16)    # [p, CT, t]
    z4 = singles.tile([128, 2, 4, 8, 256], BF16)
    zout = singles.tile([128, 16, 16, 32], F32)  # [part, chunk, t16, j]
    out4 = out.rearrange("p (ch f) -> p ch f", ch=16)
    def _stage1(tq):
        # ---- stage 1 for this quarter (all col tiles) --------------------
        t0q = tq * 256
        for CT in range(8):
            ps = psum1.tile([128, 256], F32)
            for d in range(8):
                nc.tensor.matmul(
                    ps[:],
                    lhsT=wp_sb[:, CT, d, :],
                    rhs=xt_sb[:, d, t0q:t0q + 256],
                    start=(d == 0),
                    stop=(d == 7),
                )
            nc.scalar.activation(
                y_sb[:, CT, t0q:t0q + 256], ps[:],
                mybir.ActivationFunctionType.Copy,
            )
        # ---- shuffle this quarter into z row group tq --------------------
        r = tq
        for m in range(8):
            eng = nc.sync if (m % 2 == 0) else nc.gpsimd
            eng.dma_start(
                out=z4[32 * r:32 * r + 16, :, :, m, :],
                in_=y_sb[16 * m:16 * m + 16, :, t0q:t0q + 256],
            )

        # ---- stage 2 + rms for the two 128-token chunks of this quarter --
    def _stage2(tq):
        r = tq
        for half in range(4):
            ch = 4 * tq + half
            t0 = 64 * half
            ps2 = psum2.tile([128, 16, 32], F32)
            for tw in range(64):
                c, t32 = tw % 4, tw // 4
                t256 = t0 + tw
                nc.tensor.matmul(
                    ps2[32 * c:32 * c + 32, t32, :],
                    lhsT=z4[32 * r:32 * r + 16, 0, :, :, t256],
                    rhs=z4[32 * r:32 * r + 16, 1, :, :, t256],
                    start=True, stop=True,
                    tile_position=(32 * r, 32 * c),
                )
            sq = scratch.tile([128, 16, 32], F32)
            nc.scalar.square(sq[:], ps2[:])
            part = smalls.tile([128, 16], F32R)
            with nc.allow_low_precision(reason="f32r sum of 32 sq for rms"):
                nc.vector.tensor_reduce(part[:], sq[:],
                                        axis=mybir.AxisListType.X,
                                        op=mybir.AluOpType.add)
            ps3 = psum3.tile([128, 16], F32)
            nc.tensor.matmul(ps3[:], lhsT=ind_sb[:], rhs=part[:],
                             start=True, stop=True)
            s_sb = smalls.tile([128, 16], F32)
            nc.scalar.activation(s_sb[:], ps3[:],
                                 mybir.ActivationFunctionType.Sqrt,
                                 bias=eps_sb[:], scale=1.0 / 1024.0)
            rstd = smalls.tile([128, 16], F32)
            nc.vector.reciprocal(rstd[:], s_sb[:])
            nc.vector.tensor_mul(zout[:, ch], ps2[:],
                                 rstd[:].unsqueeze(2).broadcast_to([128, 16, 32]))
            # store: device layout [32c+i, ch, t32, j]; host unpermutes
            nc.sync.dma_start(
                out=out4[:, ch, :],
                in_=zout[:, ch].rearrange("p a b -> p (a b)"),
            )



    for tq in range(4):
        _stage1(tq)
        if tq > 0:
            _stage2(tq - 1)
    _stage2(3)

@functools.lru_cache(maxsize=1)
def _build_program():
    nc = bacc.Bacc("TRN2", target_bir_lowering=False, debug=False)
    xt = nc.dram_tensor("xt", [128, 8, 1024], BF16, kind="ExternalInput").ap()
    wp = nc.dram_tensor("wp", [128, 8, 8, 128], BF16, kind="ExternalInput").ap()
    ind = nc.dram_tensor("ind", [128, 128], F32R, kind="ExternalInput").ap()
    out = nc.dram_tensor("out", [128, 8192], F32, kind="ExternalOutput").ap()
    with tile.TileContext(nc) as tc:
        with ExitStack() as ctx:
            _kernel_body(tc, ctx, xt, wp, ind, out)
    nc.compile()
    return nc


def _host_prep(x, weight):
    xf = np.ascontiguousarray(x.reshape(-1, D))          # [8192, 1024]
    # Wp column order: col = 512*sel + 128*ctp + 16*m + h ; i = 8*ctp + m
    w = weight.transpose(1, 0, 2).reshape(D, H, 2, 4, 8)  # [d, h, sel, ctp, m]
    wp = w.transpose(0, 2, 3, 4, 1).reshape(D, 1024)      # [d, col]
    wp_sb = np.ascontiguousarray(
        wp.reshape(8, 128, 8, 128).transpose(1, 2, 0, 3)).astype(
            ml_dtypes.bfloat16)
    ind = np.kron(np.eye(4, dtype=np.float32),
                  np.ones((32, 32), dtype=np.float32))
    xt_shards = []
    for c in range(N_CORES):
        xt = xf[c * T_CORE:(c + 1) * T_CORE].T            # [d, t]
        xt_sb = np.ascontiguousarray(
            xt.reshape(8, 128, 1024).transpose(1, 0, 2)).astype(
                ml_dtypes.bfloat16)
        xt_shards.append(xt_sb)
    return xt_shards, wp_sb, ind


def kernel(x, weight, **_unused):
    x = np.asarray(x, dtype=np.float32)
    weight = np.asarray(weight, dtype=np.float32)
    xt_shards, wp_sb, ind = _host_prep(x, weight)
    nc = _build_program()
    in_maps = [{"xt": xt_shards[c], "wp": wp_sb, "ind": ind}
               for c in range(N_CORES)]
    res = run_bass_kernel_spmd(nc, in_maps, list(range(N_CORES)))
    outs = []
    for c in range(N_CORES):
        d = np.asarray(res.results[c]["out"]).reshape(4, 32, 16, 16, 32)
        # [cg, i, ch, t32, j] -> token t = 128*ch + 4*t32 + cg, row = i*32+j
        outs.append(d.transpose(2, 3, 0, 1, 4).reshape(T_CORE, 1024))
    full = np.concatenate(outs, axis=0)                   # [8192, 1024]
    return full.reshape(x.shape[0], x.shape[1], 1024).astype(np.float32)


if __name__ == "__main__":
    rng = np.random.default_rng(0)
    x = rng.standard_normal((4, 2048, D), dtype=np.float32)
    w = (rng.standard_normal((H, D, 64), dtype=np.float32)
         * np.sqrt(2.0 / (D + 64))).astype(np.float32)
    o = kernel(x, w)
    print(o.shape, o.dtype)

